# revision 47
# baseline (speedup 1.0000x reference)
"""Trainium2 Bass kernel for nn_CLIP_GCN_Model (2-layer GCN + MLP + contrastive loss).

Reformulation (validated numerically, rel err ~5e-6 vs fp32 reference):
  out = mean_i(label_i * (lse_i - logits_ii)) + 1.0
(the triplet term of the reference is identically 1.0).

GCN layer: out = S @ (x @ W) + b where S = D^-1/2 (A+I) D^-1/2.
  Layer 1 (512 -> 256): compute xw1 = x @ W_g1 first, then aggregate (gather 256-wide rows).
  Layer 2 (256 -> 512): aggregate h first (gather 256-wide rows), then apply W_g2.
Aggregation: edges (incl. self-loops) sorted by dst, chunked into 128-dst-node chunks;
per 128-edge tile a coefficient matrix C [128e, 128dst] (bf16, built on host) so that
agg_chunk = sum_j C_j.T @ gathered_rows_j  (TensorE matmuls accumulating in PSUM).
Gathers via dma_gather spread over 4 SWDGE queues.

Sharding: 80 dst-chunks / 8 cores = 10 chunks per core. xw1 + image MLP replicated on
all cores (cheap). One AllGather (h) between the layers, its bubble filled by the MLP.
The contrastive rows are owned by the core holding the row's label node, so txt comes
from the core-local gcn_out slice (no second collective). Each core's image columns are
permuted so its owned rows' diagonal logits land at local column == local row index.
"""

import os
import numpy as np
import ml_dtypes

BF16 = ml_dtypes.bfloat16

N_NODES = 10000
NPAD = 10240
D = 512
Hdim = 256
BATCH = 4096
NCORES = 8
P = 128
NCHUNK = NPAD // P          # 80
CPC = NCHUNK // NCORES      # 10 chunks per core
NPC = NPAD // NCORES        # 1280 nodes per core
NT = BATCH // 512           # 8 column tiles of 512


def _wrap16(idx, n):
    """Layout indices for dma_gather: element i -> [i%16, i//16], replicated to 128 partitions."""
    assert len(idx) == n and n % 16 == 0
    base = idx.astype(np.int16).reshape(n // 16, 16).T  # [16, n/16]
    return np.ascontiguousarray(np.tile(base, (8, 1)))  # [128, n/16]


def _prep(inputs):
    """Host-side layout/sharding prep. Returns (shared, percore, T_MAX, RT)."""
    x = np.ascontiguousarray(np.asarray(inputs["x_nodes"], dtype=np.float32))
    image = np.ascontiguousarray(np.asarray(inputs["image"], dtype=np.float32))
    ei = np.asarray(inputs["edge_index"]).astype(np.int64)
    label = np.asarray(inputs["label"]).astype(np.int64)
    src, dst = ei[0], ei[1]

    deg = np.ones(N_NODES, np.float32)
    np.add.at(deg, dst, 1.0)
    dinv = (1.0 / np.sqrt(deg)).astype(np.float32)

    # edges + self loops, sorted by dst
    src_all = np.concatenate([src, np.arange(N_NODES)])
    dst_all = np.concatenate([dst, np.arange(N_NODES)])
    coef_all = np.concatenate([dinv[src] * dinv[dst], dinv * dinv]).astype(np.float32)
    order = np.argsort(dst_all, kind="stable")
    src_s, dst_s, coef_s = src_all[order], dst_all[order], coef_all[order]

    counts = np.bincount(dst_s // P, minlength=NCHUNK)
    T_MAX = int(np.ceil(counts.max() / P))
    E_c = T_MAX * P
    starts = np.zeros(NCHUNK + 1, np.int64)
    np.cumsum(counts, out=starts[1:])

    gidx = np.zeros((NCHUNK, E_c), np.int64)
    ldst = np.zeros((NCHUNK, E_c), np.int64)
    cval = np.zeros((NCHUNK, E_c), np.float32)
    for c in range(NCHUNK):
        e0, e1 = starts[c], starts[c + 1]
        k = e1 - e0
        gidx[c, :k] = src_s[e0:e1]
        ldst[c, :k] = dst_s[e0:e1] - c * P
        cval[c, :k] = coef_s[e0:e1]

    # C[c, j, p, dloc] = coef of edge (c, j*128+p) one-hot at its local dst
    C = np.zeros((NCHUNK, T_MAX, P, P), BF16)
    jj = np.arange(E_c) // P
    pp = np.arange(E_c) % P
    for c in range(NCHUNK):
        C[c, jj, pp, ldst[c]] = cval[c].astype(BF16)

    xpad = np.zeros((NPAD, D), np.float32)
    xpad[:N_NODES] = x
    xrow = np.ascontiguousarray(xpad).astype(BF16)  # [10240, 512] row-major

    def km(w, kt):  # [K, M] -> [128p, kt, M]
        return np.ascontiguousarray(
            w.reshape(kt, P, w.shape[1]).transpose(1, 0, 2)
        ).astype(BF16)

    shared = {
        "xrow": xrow,                                           # [10240, 512] bf16
        "wg1": km(np.asarray(inputs["W_g1"], np.float32), 4),   # [128, 4, 256]
        "wg2": km(np.asarray(inputs["W_g2"], np.float32), 2),   # [128, 2, 512]
        "wi1": np.ascontiguousarray(
            np.asarray(inputs["W_img1"], np.float32).reshape(4, P, 2, P).transpose(1, 0, 2, 3)
        ).astype(BF16),                                         # [128, 4k, 2m, 128]
        "wi2": np.ascontiguousarray(
            np.asarray(inputs["W_img2"], np.float32).reshape(2, P, 4, P).transpose(1, 0, 2, 3)
        ).astype(BF16),                                         # [128, 2k, 4m, 128]
        "bg1": np.asarray(inputs["b_g1"], np.float32).astype(BF16).reshape(1, Hdim),
        "bg2": np.asarray(inputs["b_g2"], np.float32).astype(BF16).reshape(1, D),
        "bi1": np.ascontiguousarray(np.asarray(inputs["b_img1"], np.float32).reshape(2, P).T),
        "bi2": np.ascontiguousarray(np.asarray(inputs["b_img2"], np.float32).reshape(4, P).T),
    }

    # owner-core logits rows: core c owns batch rows whose label node is in its slice
    owner = label // NPC
    rows_by_core = [np.where(owner == c)[0] for c in range(NCORES)]
    RT = max(2, int(np.ceil(max(len(r) for r in rows_by_core) / P)))
    ROWS = RT * P

    percore = []
    imageb = image.astype(BF16)
    for c in range(NCORES):
        Cc = np.ascontiguousarray(
            C[c * CPC:(c + 1) * CPC].transpose(2, 0, 1, 3)
        )  # [128p, 10, T_MAX, 128]
        gi = np.zeros((P, CPC, E_c // 16), np.int16)
        gi2 = np.zeros((P, CPC, E_c // 16), np.int16)
        for i in range(CPC):
            idxs = gidx[c * CPC + i]
            gi[:, i, :] = _wrap16(idxs, E_c)
            # h table rows are permuted: [AG-half][core][chunk%5][row]
            oc = idxs // NPC
            oi = (idxs % NPC) // P
            orr = idxs % P
            remap = (oi >= CPC // 2) * (NPAD // 2) + oc * (NPC // 2) + (oi % (CPC // 2)) * P + orr
            gi2[:, i, :] = _wrap16(remap, E_c)
        rows = rows_by_core[c]
        n_c = len(rows)
        loc = np.zeros(ROWS, np.int64)
        loc[:n_c] = label[rows] - c * NPC          # local row in g_own
        li = _wrap16(loc, ROWS)
        lab_f = np.zeros(ROWS, np.float32)
        lab_f[:n_c] = label[rows].astype(np.float32)
        lab_f = np.ascontiguousarray(lab_f.reshape(RT, P).T)   # [128, RT]
        # per-core image column permutation: owned rows first, then the rest
        rest = np.setdiff1d(np.arange(BATCH), rows, assume_unique=True)
        perm = np.concatenate([rows, rest])
        imr = imageb[perm]
        imt = np.ascontiguousarray(
            imr.reshape(NT, 512, 4, P).transpose(0, 3, 2, 1)
        )  # [8, 128p, 4k, 512w]
        percore.append({"cmat": Cc, "gidx": gi, "gidx2": gi2, "lidx": li,
                        "labf": lab_f, "imt": imt})
    return shared, percore, T_MAX, RT


def _build(T_MAX, RT):
    """Build the SPMD Bass program."""
    import concourse.bass as bass  # noqa: F401
    import concourse.tile as tile
    from concourse import bacc, mybir
    from concourse.masks import make_identity

    fp32 = mybir.dt.float32
    bf16 = mybir.dt.bfloat16
    i16 = mybir.dt.int16
    AF = mybir.ActivationFunctionType
    AX = mybir.AxisListType
    E_c = T_MAX * P
    ROWS = RT * P

    nc = bacc.Bacc("TRN2", target_bir_lowering=False, debug=False,
                   num_devices=NCORES, num_swdge_queues=4)

    t_xrow = nc.dram_tensor("xrow", [NPAD, D], bf16, kind="ExternalInput").ap()
    t_wg1 = nc.dram_tensor("wg1", [P, 4, Hdim], bf16, kind="ExternalInput").ap()
    t_wg2 = nc.dram_tensor("wg2", [P, 2, D], bf16, kind="ExternalInput").ap()
    t_wi1 = nc.dram_tensor("wi1", [P, 4, 2, P], bf16, kind="ExternalInput").ap()
    t_wi2 = nc.dram_tensor("wi2", [P, 2, 4, P], bf16, kind="ExternalInput").ap()
    t_bg1 = nc.dram_tensor("bg1", [1, Hdim], bf16, kind="ExternalInput").ap()
    t_bg2 = nc.dram_tensor("bg2", [1, D], bf16, kind="ExternalInput").ap()
    t_bi1 = nc.dram_tensor("bi1", [P, 2], fp32, kind="ExternalInput").ap()
    t_bi2 = nc.dram_tensor("bi2", [P, 4], fp32, kind="ExternalInput").ap()
    t_cmat = nc.dram_tensor("cmat", [P, CPC, T_MAX, P], bf16, kind="ExternalInput").ap()
    t_gidx = nc.dram_tensor("gidx", [P, CPC, E_c // 16], i16, kind="ExternalInput").ap()
    t_gidx2 = nc.dram_tensor("gidx2", [P, CPC, E_c // 16], i16, kind="ExternalInput").ap()
    t_lidx = nc.dram_tensor("lidx", [P, ROWS // 16], i16, kind="ExternalInput").ap()
    t_labf = nc.dram_tensor("labf", [P, RT], fp32, kind="ExternalInput").ap()
    t_imt = nc.dram_tensor("imt", [NT, P, 4, 512], bf16, kind="ExternalInput").ap()
    t_out = nc.dram_tensor("partial", [1, 1], fp32, kind="ExternalOutput").ap()

    rg = [list(range(NCORES))]

    with tile.TileContext(nc) as tc:
        from contextlib import ExitStack
        with ExitStack() as ctx:
            dram = ctx.enter_context(tc.tile_pool(name="dram", bufs=1, space="DRAM"))
            const = ctx.enter_context(tc.tile_pool(name="const", bufs=1))
            big = ctx.enter_context(tc.tile_pool(name="big", bufs=1))
            work = ctx.enter_context(tc.tile_pool(name="work", bufs=3))
            gbuf = ctx.enter_context(tc.tile_pool(name="gbuf", bufs=2))
            stat = ctx.enter_context(tc.tile_pool(name="stat", bufs=4))

            h_own = dram.tile([NPC, Hdim], bf16)
            h_t = dram.tile([NPAD, Hdim], bf16)
            g_own = dram.tile([NPC, D], bf16)

            # ---- constants in SBUF ----
            wg1_s = const.tile([P, 4, Hdim], bf16)
            nc.sync.dma_start(out=wg1_s[:], in_=t_wg1[:])
            wg2_s = const.tile([P, 2, D], bf16)
            nc.sync.dma_start(out=wg2_s[:], in_=t_wg2[:])
            wi1_s = const.tile([P, 4, 2, P], bf16)
            nc.sync.dma_start(out=wi1_s[:], in_=t_wi1[:])
            wi2_s = const.tile([P, 2, 4, P], bf16)
            nc.sync.dma_start(out=wi2_s[:], in_=t_wi2[:])
            bg1_s = const.tile([1, Hdim], bf16)
            nc.sync.dma_start(out=bg1_s[:], in_=t_bg1[:])
            bg2_s = const.tile([1, D], bf16)
            nc.sync.dma_start(out=bg2_s[:], in_=t_bg2[:])
            bi1_s = const.tile([P, 2], fp32)
            nc.sync.dma_start(out=bi1_s[:], in_=t_bi1[:])
            bi2_s = const.tile([P, 4], fp32)
            nc.sync.dma_start(out=bi2_s[:], in_=t_bi2[:])
            labf_s = const.tile([P, RT], fp32)
            nc.sync.dma_start(out=labf_s[:], in_=t_labf[:])
            lidx_s = const.tile([P, ROWS // 16], i16)
            nc.sync.dma_start(out=lidx_s[:], in_=t_lidx[:])
            gidx_s = const.tile([P, CPC, E_c // 16], i16)
            nc.sync.dma_start(out=gidx_s[:], in_=t_gidx[:])
            gidx2_s = const.tile([P, CPC, E_c // 16], i16)
            nc.sync.dma_start(out=gidx2_s[:], in_=t_gidx2[:])
            ones_row = const.tile([1, P], bf16)
            nc.vector.memset(ones_row[:], 1.0)
            ones_col = const.tile([P, 1], fp32)
            nc.vector.memset(ones_col[:], 1.0)
            ident_b = const.tile([P, P], bf16)
            make_identity(nc, ident_b[:])
            ident_f = const.tile([P, P], fp32)
            make_identity(nc, ident_f[:])

            imgT_s = big.tile([P, 4, BATCH], bf16)  # transposed img (permuted cols)

            # ===== GCN layer 1 (my 10 chunks): gather raw x rows, W after agg =
            # split each chunk's gather (and its matmuls) in halves for finer
            # pipelining across the 4 SWDGE queues.
            TH = T_MAX // 2
            TH2 = T_MAX - TH
            HALF = ((0, TH), (TH, T_MAX))

            def l1_chunk(i, ps_ag):
                cm = gbuf.tile([P, T_MAX, P], bf16, tag="cm", name="cm")
                nc.scalar.dma_start(out=cm[:], in_=t_cmat[:, i, :, :])
                pa = ps_ag.tile([P, D], fp32, tag="agg1", name="pa")
                ghs = []
                for hf, (j0, j1) in enumerate(HALF):
                    gh = gbuf.tile([P, j1 - j0, D], bf16, tag=f"g1_{hf}", name="gh")
                    nc.gpsimd.dma_gather(
                        out_ap=gh[:], in_ap=t_xrow[:, :],
                        idxs_ap=gidx_s[:, i, j0 * 8:j1 * 8],
                        num_idxs=(j1 - j0) * P, num_idxs_reg=(j1 - j0) * P,
                        elem_size=D, single_packet=False,
                        queue_num=(2 * i + hf) % 4,
                    )
                    ghs.append(gh)
                for hf, (j0, j1) in enumerate(HALF):
                    for j in range(j0, j1):
                        nc.tensor.matmul(
                            out=pa[:], lhsT=cm[:, j, :], rhs=ghs[hf][:, j - j0, :],
                            start=(j == 0), stop=(j == T_MAX - 1),
                        )
                a1 = work.tile([P, D], bf16, tag="a1", name="a1")
                nc.vector.tensor_copy(out=a1[:], in_=pa[:])
                a1t = work.tile([P, 4, P], bf16, tag="a1t", name="a1t")
                for k in range(4):
                    pt1 = ps_ag.tile([P, P], bf16, tag="tps1", name="pt1")
                    nc.tensor.transpose(
                        out=pt1[:], in_=a1[:, k * P:(k + 1) * P], identity=ident_b[:]
                    )
                    nc.vector.tensor_copy(out=a1t[:, k, :], in_=pt1[:])
                ph = ps_ag.tile([P, Hdim], fp32, tag="hps", name="ph")
                for k in range(4):
                    nc.tensor.matmul(
                        out=ph[:], lhsT=a1t[:, k, :], rhs=wg1_s[:, k, :],
                        start=(k == 0), stop=False,
                    )
                nc.tensor.matmul(
                    out=ph[:], lhsT=ones_row[:], rhs=bg1_s[:],
                    start=False, stop=True, skip_group_check=True,
                )
                h_sb = work.tile([P, Hdim], bf16, tag="h_sb", name="h_sb")
                nc.scalar.activation(out=h_sb[:], in_=ph[:], func=AF.Relu)
                nc.sync.dma_start(out=h_own[i * P:(i + 1) * P, :], in_=h_sb[:])

            CH1 = CPC // 2  # first 5 chunks, then AllGather half 1
            with tc.tile_pool(name="ps_ag", bufs=2, space="PSUM") as ps_ag:
                for i in range(CH1):
                    l1_chunk(i, ps_ag)
                nc.gpsimd.collective_compute(
                    "AllGather", mybir.AluOpType.bypass, replica_groups=rg,
                    ins=[h_own[0:CH1 * P, :]], outs=[h_t[0:NPAD // 2, :]],
                )
                for i in range(CH1, CPC):
                    l1_chunk(i, ps_ag)
                nc.gpsimd.collective_compute(
                    "AllGather", mybir.AluOpType.bypass, replica_groups=rg,
                    ins=[h_own[CH1 * P:NPC, :]], outs=[h_t[NPAD // 2:NPAD, :]],
                )

            # ====== image MLP (emitted here so it fills the AllGather bubble) =
            with tc.tile_pool(name="ps_mlp", bufs=2, space="PSUM") as ps_mlp:
                for n in range(NT):
                    imt_n = work.tile([P, 4, 512], bf16, tag="imt_n")
                    nc.sync.dma_start(out=imt_n[:], in_=t_imt[n])
                    h1t = work.tile([P, 2, 512], bf16, tag="h1t")
                    for m in range(2):
                        pm = ps_mlp.tile([P, 512], fp32, tag="mlp1")
                        for k in range(4):
                            nc.tensor.matmul(
                                out=pm[:], lhsT=wi1_s[:, k, m, :], rhs=imt_n[:, k, :],
                                start=(k == 0), stop=(k == 3),
                            )
                        nc.scalar.activation(
                            out=h1t[:, m, :], in_=pm[:], func=AF.Relu,
                            bias=bi1_s[:, m:m + 1], scale=1.0,
                        )
                    for m in range(4):
                        pm2 = ps_mlp.tile([P, 512], fp32, tag="mlp2")
                        for k in range(2):
                            nc.tensor.matmul(
                                out=pm2[:], lhsT=wi2_s[:, k, m, :], rhs=h1t[:, k, :],
                                start=(k == 0), stop=(k == 1),
                            )
                        nc.scalar.activation(
                            out=imgT_s[:, m, n * 512:(n + 1) * 512], in_=pm2[:],
                            func=AF.Relu, bias=bi2_s[:, m:m + 1], scale=1.0,
                        )

            # ================= GCN layer 2 (my 10 chunks) =====================
            with tc.tile_pool(name="ps_l2", bufs=2, space="PSUM") as ps_l2:
                for i in range(CPC):
                    cm2 = gbuf.tile([P, T_MAX, P], bf16, tag="cm", name="cm2")
                    nc.scalar.dma_start(out=cm2[:], in_=t_cmat[:, i, :, :])
                    pa2 = ps_l2.tile([P, Hdim], fp32, tag="agg2", name="pa2")
                    g2s = []
                    for hf, (j0, j1) in enumerate(HALF):
                        g2h = gbuf.tile([P, j1 - j0, Hdim], bf16, tag=f"g2_{hf}", name="g2h")
                        nc.gpsimd.dma_gather(
                            out_ap=g2h[:], in_ap=h_t[:, :],
                            idxs_ap=gidx2_s[:, i, j0 * 8:j1 * 8],
                            num_idxs=(j1 - j0) * P, num_idxs_reg=(j1 - j0) * P,
                            elem_size=Hdim, single_packet=False,
                            queue_num=(2 * i + hf) % 4,
                        )
                        g2s.append(g2h)
                    for hf, (j0, j1) in enumerate(HALF):
                        for j in range(j0, j1):
                            nc.tensor.matmul(
                                out=pa2[:], lhsT=cm2[:, j, :], rhs=g2s[hf][:, j - j0, :],
                                start=(j == 0), stop=(j == T_MAX - 1),
                            )
                    a2 = work.tile([P, Hdim], bf16, tag="a2")
                    nc.vector.tensor_copy(out=a2[:], in_=pa2[:])
                    a2t = work.tile([P, 2, P], bf16, tag="a2t")
                    for k in range(2):
                        pt = ps_l2.tile([P, P], bf16, tag="tps")
                        nc.tensor.transpose(
                            out=pt[:], in_=a2[:, k * P:(k + 1) * P], identity=ident_b[:]
                        )
                        nc.vector.tensor_copy(out=a2t[:, k, :], in_=pt[:])
                    pg = ps_l2.tile([P, D], fp32, tag="outg")
                    for k in range(2):
                        nc.tensor.matmul(
                            out=pg[:], lhsT=a2t[:, k, :], rhs=wg2_s[:, k, :],
                            start=(k == 0), stop=False,
                        )
                    nc.tensor.matmul(
                        out=pg[:], lhsT=ones_row[:], rhs=bg2_s[:],
                        start=False, stop=True, skip_group_check=True,
                    )
                    g_sb = work.tile([P, D], bf16, tag="g_sb")
                    nc.vector.tensor_copy(out=g_sb[:], in_=pg[:])
                    nc.sync.dma_start(out=g_own[i * P:(i + 1) * P, :], in_=g_sb[:])

            # ================= txt gather (core-local) + transpose ============
            txt = big.tile([P, RT, D], bf16)
            nc.gpsimd.dma_gather(
                out_ap=txt[:], in_ap=g_own[:, :], idxs_ap=lidx_s[:],
                num_idxs=ROWS, num_idxs_reg=ROWS, elem_size=D,
                single_packet=False, queue_num=0,
            )
            txtT = big.tile([P, RT, 4, P], bf16)  # [p, r, k, rows]
            with tc.tile_pool(name="ps_tt", bufs=2, space="PSUM") as ps_tt:
                for r in range(RT):
                    for k in range(4):
                        ptt = ps_tt.tile([P, P], bf16, tag="ttps")
                        nc.tensor.transpose(
                            out=ptt[:], in_=txt[:, r, k * P:(k + 1) * P],
                            identity=ident_b[:],
                        )
                        nc.vector.tensor_copy(out=txtT[:, r, k, :], in_=ptt[:])

            # ================= logits + row losses ============================
            contrib = stat.tile([P, RT], fp32)
            with tc.tile_pool(name="ps_lg", bufs=8, space="PSUM") as ps_lg:
                for r in range(RT):
                    banks = []
                    for n in range(NT):
                        pl = ps_lg.tile([P, 512], fp32, tag="lg")
                        for k in range(4):
                            nc.tensor.matmul(
                                out=pl[:], lhsT=txtT[:, r, k, :],
                                rhs=imgT_s[:, k, n * 512:(n + 1) * 512],
                                start=(k == 0), stop=(k == 3),
                            )
                        banks.append(pl)
                    maxes = stat.tile([P, NT], fp32, tag="maxes")
                    for n in range(NT):
                        nc.vector.reduce_max(out=maxes[:, n:n + 1], in_=banks[n][:], axis=AX.X)
                    rmax = stat.tile([P, 1], fp32, tag="rmax")
                    nc.vector.reduce_max(out=rmax[:], in_=maxes[:], axis=AX.X)
                    nrmax = stat.tile([P, 1], fp32, tag="nrmax")
                    nc.scalar.mul(nrmax[:], rmax[:], -1.0)
                    # diag block for row-tile r is at local cols r*128..r*128+127
                    br, off = (r * P) // 512, (r * P) % 512
                    dtmp = stat.tile([P, P], fp32, tag="dtmp")
                    nc.vector.tensor_tensor(
                        out=dtmp[:], in0=banks[br][:, off:off + P], in1=ident_f[:],
                        op=mybir.AluOpType.mult,
                    )
                    diag = stat.tile([P, 1], fp32, tag="diag")
                    nc.vector.reduce_sum(out=diag[:], in_=dtmp[:], axis=AX.X)
                    sums = stat.tile([P, NT], fp32, tag="sums")
                    for n in range(NT):
                        esc = work.tile([P, 512], fp32, tag="esc")
                        nc.scalar.activation(
                            out=esc[:], in_=banks[n][:], func=AF.Exp,
                            bias=nrmax[:], scale=1.0, accum_out=sums[:, n:n + 1],
                        )
                    ssum = stat.tile([P, 1], fp32, tag="ssum")
                    nc.vector.reduce_sum(out=ssum[:], in_=sums[:], axis=AX.X)
                    lns = stat.tile([P, 1], fp32, tag="lns")
                    nc.scalar.activation(out=lns[:], in_=ssum[:], func=AF.Ln)
                    t1 = stat.tile([P, 1], fp32, tag="t1")
                    nc.vector.tensor_add(out=t1[:], in0=rmax[:], in1=lns[:])
                    nc.vector.tensor_sub(out=t1[:], in0=t1[:], in1=diag[:])
                    nc.vector.tensor_mul(
                        out=contrib[:, r:r + 1], in0=t1[:], in1=labf_s[:, r:r + 1]
                    )
            rsum = stat.tile([P, 1], fp32, tag="rsum")
            nc.vector.reduce_sum(out=rsum[:], in_=contrib[:], axis=AX.X)
            with tc.tile_pool(name="ps_fin", bufs=1, space="PSUM") as ps_fin:
                pf = ps_fin.tile([1, 1], fp32)
                nc.tensor.matmul(out=pf[:], lhsT=rsum[:], rhs=ones_col[:], start=True, stop=True)
                fin = stat.tile([1, 1], fp32, tag="fin")
                nc.vector.tensor_copy(out=fin[:], in_=pf[:])
            nc.sync.dma_start(out=t_out[:], in_=fin[:])

    nc.compile()
    return nc


_CACHE = {}


def kernel(**inputs) -> np.ndarray:
    from concourse.bass_utils import run_bass_kernel_spmd

    shared, percore, T_MAX, RT = _prep(inputs)
    key = (T_MAX, RT)
    if key not in _CACHE:
        _CACHE[key] = _build(T_MAX, RT)
    nc = _CACHE[key]

    in_maps = []
    for c in range(NCORES):
        m = {
            "xrow": shared["xrow"], "wg1": shared["wg1"], "wg2": shared["wg2"],
            "wi1": shared["wi1"], "wi2": shared["wi2"],
            "bg1": shared["bg1"], "bg2": shared["bg2"],
            "bi1": shared["bi1"], "bi2": shared["bi2"],
            "cmat": percore[c]["cmat"], "gidx": percore[c]["gidx"],
            "gidx2": percore[c]["gidx2"], "lidx": percore[c]["lidx"],
            "labf": percore[c]["labf"], "imt": percore[c]["imt"],
        }
        in_maps.append(m)

    trace = bool(int(os.environ.get("KERNEL_TRACE", "0")))
    try:
        res = run_bass_kernel_spmd(nc, in_maps, core_ids=list(range(NCORES)), trace=trace)
    except Exception:
        # transient NRT/device hiccups have been observed to clear on retry
        res = run_bass_kernel_spmd(nc, in_maps, core_ids=list(range(NCORES)), trace=trace)
    kernel.last_results = res
    total = sum(float(r["partial"][0, 0]) for r in res.results)
    return np.float32(total / BATCH + 1.0)


# revision 48
# speedup vs baseline: 1.0992x; 1.0992x over previous
"""Trainium2 Bass kernel for nn_CLIP_GCN_Model (2-layer GCN + MLP + contrastive loss).

Reformulation (validated numerically, rel err ~5e-6 vs fp32 reference):
  out = mean_i(label_i * (lse_i - logits_ii)) + 1.0
(the triplet term of the reference is identically 1.0).

GCN layer: out = S @ (x @ W) + b where S = D^-1/2 (A+I) D^-1/2.
  Layer 1 (512 -> 256): compute xw1 = x @ W_g1 first, then aggregate (gather 256-wide rows).
  Layer 2 (256 -> 512): aggregate h first (gather 256-wide rows), then apply W_g2.
Aggregation: edges (incl. self-loops) sorted by dst, chunked into 128-dst-node chunks;
per 128-edge tile a coefficient matrix C [128e, 128dst] (bf16, built on host) so that
agg_chunk = sum_j C_j.T @ gathered_rows_j  (TensorE matmuls accumulating in PSUM).
Gathers via dma_gather spread over 4 SWDGE queues.

Sharding: 80 dst-chunks / 8 cores = 10 chunks per core. xw1 + image MLP replicated on
all cores (cheap). One AllGather (h) between the layers, its bubble filled by the MLP.
The contrastive rows are owned by the core holding the row's label node, so txt comes
from the core-local gcn_out slice (no second collective). Each core's image columns are
permuted so its owned rows' diagonal logits land at local column == local row index.
"""

import os
import numpy as np
import ml_dtypes

BF16 = ml_dtypes.bfloat16

N_NODES = 10000
NPAD = 10240
D = 512
Hdim = 256
BATCH = 4096
NCORES = 8
P = 128
NCHUNK = NPAD // P          # 80
CPC = NCHUNK // NCORES      # 10 chunks per core
NPC = NPAD // NCORES        # 1280 nodes per core
NT = BATCH // 512           # 8 column tiles of 512


def _wrap16(idx, n):
    """Layout indices for dma_gather: element i -> [i%16, i//16], replicated to 128 partitions."""
    assert len(idx) == n and n % 16 == 0
    base = idx.astype(np.int16).reshape(n // 16, 16).T  # [16, n/16]
    return np.ascontiguousarray(np.tile(base, (8, 1)))  # [128, n/16]


def _prep(inputs):
    """Host-side layout/sharding prep. Returns (shared, percore, T_MAX, RT)."""
    x = np.ascontiguousarray(np.asarray(inputs["x_nodes"], dtype=np.float32))
    image = np.ascontiguousarray(np.asarray(inputs["image"], dtype=np.float32))
    ei = np.asarray(inputs["edge_index"]).astype(np.int64)
    label = np.asarray(inputs["label"]).astype(np.int64)
    src, dst = ei[0], ei[1]

    deg = np.ones(N_NODES, np.float32)
    np.add.at(deg, dst, 1.0)
    dinv = (1.0 / np.sqrt(deg)).astype(np.float32)

    # edges + self loops, sorted by dst
    src_all = np.concatenate([src, np.arange(N_NODES)])
    dst_all = np.concatenate([dst, np.arange(N_NODES)])
    coef_all = np.concatenate([dinv[src] * dinv[dst], dinv * dinv]).astype(np.float32)
    order = np.argsort(dst_all, kind="stable")
    src_s, dst_s, coef_s = src_all[order], dst_all[order], coef_all[order]

    counts = np.bincount(dst_s // P, minlength=NCHUNK)
    T_MAX = int(np.ceil(counts.max() / P))
    E_c = T_MAX * P
    starts = np.zeros(NCHUNK + 1, np.int64)
    np.cumsum(counts, out=starts[1:])

    gidx = np.zeros((NCHUNK, E_c), np.int64)
    ldst = np.zeros((NCHUNK, E_c), np.int64)
    cval = np.zeros((NCHUNK, E_c), np.float32)
    for c in range(NCHUNK):
        e0, e1 = starts[c], starts[c + 1]
        k = e1 - e0
        gidx[c, :k] = src_s[e0:e1]
        ldst[c, :k] = dst_s[e0:e1] - c * P
        cval[c, :k] = coef_s[e0:e1]

    # C[c, j, p, dloc] = coef of edge (c, j*128+p) one-hot at its local dst
    C = np.zeros((NCHUNK, T_MAX, P, P), BF16)
    jj = np.arange(E_c) // P
    pp = np.arange(E_c) % P
    for c in range(NCHUNK):
        C[c, jj, pp, ldst[c]] = cval[c].astype(BF16)

    xpad = np.zeros((NPAD, D), np.float32)
    xpad[:N_NODES] = x
    xrow = np.ascontiguousarray(xpad).astype(BF16)  # [10240, 512] row-major

    def km(w, kt):  # [K, M] -> [128p, kt, M]
        return np.ascontiguousarray(
            w.reshape(kt, P, w.shape[1]).transpose(1, 0, 2)
        ).astype(BF16)

    shared = {
        "xrow": xrow,                                           # [10240, 512] bf16
        "wg1": km(np.asarray(inputs["W_g1"], np.float32), 4),   # [128, 4, 256]
        "wg2": km(np.asarray(inputs["W_g2"], np.float32), 2),   # [128, 2, 512]
        "wi1": np.ascontiguousarray(
            np.asarray(inputs["W_img1"], np.float32).reshape(4, P, 2, P).transpose(1, 0, 2, 3)
        ).astype(BF16),                                         # [128, 4k, 2m, 128]
        "wi2": np.ascontiguousarray(
            np.asarray(inputs["W_img2"], np.float32).reshape(2, P, 4, P).transpose(1, 0, 2, 3)
        ).astype(BF16),                                         # [128, 2k, 4m, 128]
        "bg1": np.asarray(inputs["b_g1"], np.float32).astype(BF16).reshape(1, Hdim),
        "bg2": np.asarray(inputs["b_g2"], np.float32).astype(BF16).reshape(1, D),
        "bi1": np.ascontiguousarray(np.asarray(inputs["b_img1"], np.float32).reshape(2, P).T),
        "bi2": np.ascontiguousarray(np.asarray(inputs["b_img2"], np.float32).reshape(4, P).T),
    }

    # owner-core logits rows: core c owns batch rows whose label node is in its slice
    owner = label // NPC
    rows_by_core = [np.where(owner == c)[0] for c in range(NCORES)]
    RT = max(2, int(np.ceil(max(len(r) for r in rows_by_core) / P)))
    ROWS = RT * P

    percore = []
    imageb = image.astype(BF16)
    for c in range(NCORES):
        Cc = np.ascontiguousarray(
            C[c * CPC:(c + 1) * CPC].transpose(2, 0, 1, 3)
        )  # [128p, 10, T_MAX, 128]
        gi = np.zeros((P, CPC, E_c // 16), np.int16)
        gi2 = np.zeros((P, CPC, E_c // 16), np.int16)
        for i in range(CPC):
            idxs = gidx[c * CPC + i]
            gi[:, i, :] = _wrap16(idxs, E_c)
            # h table rows are permuted: [AG-half][core][chunk%5][row]
            oc = idxs // NPC
            oi = (idxs % NPC) // P
            orr = idxs % P
            remap = (oi >= CPC // 2) * (NPAD // 2) + oc * (NPC // 2) + (oi % (CPC // 2)) * P + orr
            gi2[:, i, :] = _wrap16(remap, E_c)
        rows = rows_by_core[c]
        n_c = len(rows)
        loc = np.zeros(ROWS, np.int64)
        loc[:n_c] = label[rows] - c * NPC          # local row in g_own
        li = _wrap16(loc, ROWS)
        lab_f = np.zeros(ROWS, np.float32)
        lab_f[:n_c] = label[rows].astype(np.float32)
        lab_f = np.ascontiguousarray(lab_f.reshape(RT, P).T)   # [128, RT]
        # per-core image column permutation: owned rows first, then the rest
        rest = np.setdiff1d(np.arange(BATCH), rows, assume_unique=True)
        perm = np.concatenate([rows, rest])
        imr = imageb[perm]
        imt = np.ascontiguousarray(
            imr.reshape(NT, 512, 4, P).transpose(0, 3, 2, 1)
        )  # [8, 128p, 4k, 512w]
        percore.append({"cmat": Cc, "gidx": gi, "gidx2": gi2, "lidx": li,
                        "labf": lab_f, "imt": imt})
    return shared, percore, T_MAX, RT


def _build(T_MAX, RT):
    """Build the SPMD Bass program."""
    import concourse.bass as bass  # noqa: F401
    import concourse.tile as tile
    from concourse import bacc, mybir
    from concourse.masks import make_identity

    fp32 = mybir.dt.float32
    bf16 = mybir.dt.bfloat16
    i16 = mybir.dt.int16
    AF = mybir.ActivationFunctionType
    AX = mybir.AxisListType
    E_c = T_MAX * P
    ROWS = RT * P

    nc = bacc.Bacc("TRN2", target_bir_lowering=False, debug=False,
                   num_devices=NCORES, num_swdge_queues=4)

    t_xrow = nc.dram_tensor("xrow", [NPAD, D], bf16, kind="ExternalInput").ap()
    t_wg1 = nc.dram_tensor("wg1", [P, 4, Hdim], bf16, kind="ExternalInput").ap()
    t_wg2 = nc.dram_tensor("wg2", [P, 2, D], bf16, kind="ExternalInput").ap()
    t_wi1 = nc.dram_tensor("wi1", [P, 4, 2, P], bf16, kind="ExternalInput").ap()
    t_wi2 = nc.dram_tensor("wi2", [P, 2, 4, P], bf16, kind="ExternalInput").ap()
    t_bg1 = nc.dram_tensor("bg1", [1, Hdim], bf16, kind="ExternalInput").ap()
    t_bg2 = nc.dram_tensor("bg2", [1, D], bf16, kind="ExternalInput").ap()
    t_bi1 = nc.dram_tensor("bi1", [P, 2], fp32, kind="ExternalInput").ap()
    t_bi2 = nc.dram_tensor("bi2", [P, 4], fp32, kind="ExternalInput").ap()
    t_cmat = nc.dram_tensor("cmat", [P, CPC, T_MAX, P], bf16, kind="ExternalInput").ap()
    t_gidx = nc.dram_tensor("gidx", [P, CPC, E_c // 16], i16, kind="ExternalInput").ap()
    t_gidx2 = nc.dram_tensor("gidx2", [P, CPC, E_c // 16], i16, kind="ExternalInput").ap()
    t_lidx = nc.dram_tensor("lidx", [P, ROWS // 16], i16, kind="ExternalInput").ap()
    t_labf = nc.dram_tensor("labf", [P, RT], fp32, kind="ExternalInput").ap()
    t_imt = nc.dram_tensor("imt", [NT, P, 4, 512], bf16, kind="ExternalInput").ap()
    t_out = nc.dram_tensor("partial", [1, 1], fp32, kind="ExternalOutput").ap()

    rg = [list(range(NCORES))]

    with tile.TileContext(nc) as tc:
        from contextlib import ExitStack
        with ExitStack() as ctx:
            dram = ctx.enter_context(tc.tile_pool(name="dram", bufs=1, space="DRAM"))
            const = ctx.enter_context(tc.tile_pool(name="const", bufs=1))
            big = ctx.enter_context(tc.tile_pool(name="big", bufs=1))
            work = ctx.enter_context(tc.tile_pool(name="work", bufs=3))
            gbuf = ctx.enter_context(tc.tile_pool(name="gbuf", bufs=3))
            stat = ctx.enter_context(tc.tile_pool(name="stat", bufs=4))

            h_own = dram.tile([NPC, Hdim], bf16)
            h_t = dram.tile([NPAD, Hdim], bf16)
            g_own = dram.tile([NPC, D], bf16)

            # ---- constants in SBUF ----
            wg1_s = const.tile([P, 4, Hdim], bf16)
            nc.sync.dma_start(out=wg1_s[:], in_=t_wg1[:])
            wg2_s = const.tile([P, 2, D], bf16)
            nc.sync.dma_start(out=wg2_s[:], in_=t_wg2[:])
            wi1_s = const.tile([P, 4, 2, P], bf16)
            nc.sync.dma_start(out=wi1_s[:], in_=t_wi1[:])
            wi2_s = const.tile([P, 2, 4, P], bf16)
            nc.sync.dma_start(out=wi2_s[:], in_=t_wi2[:])
            bg1_s = const.tile([1, Hdim], bf16)
            nc.sync.dma_start(out=bg1_s[:], in_=t_bg1[:])
            bg2_s = const.tile([1, D], bf16)
            nc.sync.dma_start(out=bg2_s[:], in_=t_bg2[:])
            bi1_s = const.tile([P, 2], fp32)
            nc.sync.dma_start(out=bi1_s[:], in_=t_bi1[:])
            bi2_s = const.tile([P, 4], fp32)
            nc.sync.dma_start(out=bi2_s[:], in_=t_bi2[:])
            labf_s = const.tile([P, RT], fp32)
            nc.sync.dma_start(out=labf_s[:], in_=t_labf[:])
            lidx_s = const.tile([P, ROWS // 16], i16)
            nc.sync.dma_start(out=lidx_s[:], in_=t_lidx[:])
            gidx_s = const.tile([P, CPC, E_c // 16], i16)
            nc.sync.dma_start(out=gidx_s[:], in_=t_gidx[:])
            gidx2_s = const.tile([P, CPC, E_c // 16], i16)
            nc.sync.dma_start(out=gidx2_s[:], in_=t_gidx2[:])
            ones_row = const.tile([1, P], bf16)
            nc.vector.memset(ones_row[:], 1.0)
            ones_col = const.tile([P, 1], fp32)
            nc.vector.memset(ones_col[:], 1.0)
            ident_b = const.tile([P, P], bf16)
            make_identity(nc, ident_b[:])
            ident_f = const.tile([P, P], fp32)
            make_identity(nc, ident_f[:])

            imgT_s = big.tile([P, 4, BATCH], bf16)  # transposed img (permuted cols)

            # ===== GCN layer 1 (my 10 chunks): gather raw x rows, W after agg =
            # split each chunk's gather (and its matmuls) in halves for finer
            # pipelining across the 4 SWDGE queues.
            TH = T_MAX // 2
            TH2 = T_MAX - TH
            HALF = ((0, TH), (TH, T_MAX))

            def l1_chunk(i, ps_ag):
                cm = gbuf.tile([P, T_MAX, P], bf16, tag="cm", name="cm")
                nc.scalar.dma_start(out=cm[:], in_=t_cmat[:, i, :, :])
                pa = ps_ag.tile([P, D], fp32, tag="agg1", name="pa")
                ghs = []
                for hf, (j0, j1) in enumerate(HALF):
                    gh = gbuf.tile([P, j1 - j0, D], bf16, tag=f"g1_{hf}", name="gh")
                    nc.gpsimd.dma_gather(
                        out_ap=gh[:], in_ap=t_xrow[:, :],
                        idxs_ap=gidx_s[:, i, j0 * 8:j1 * 8],
                        num_idxs=(j1 - j0) * P, num_idxs_reg=(j1 - j0) * P,
                        elem_size=D, single_packet=False,
                        queue_num=(2 * i + hf) % 4,
                    )
                    ghs.append(gh)
                for hf, (j0, j1) in enumerate(HALF):
                    for j in range(j0, j1):
                        nc.tensor.matmul(
                            out=pa[:], lhsT=cm[:, j, :], rhs=ghs[hf][:, j - j0, :],
                            start=(j == 0), stop=(j == T_MAX - 1),
                        )
                a1 = work.tile([P, D], bf16, tag="a1", name="a1")
                nc.vector.tensor_copy(out=a1[:], in_=pa[:])
                a1t = work.tile([P, 4, P], bf16, tag="a1t", name="a1t")
                for k in range(4):
                    pt1 = ps_ag.tile([P, P], bf16, tag="tps1", name="pt1")
                    nc.tensor.transpose(
                        out=pt1[:], in_=a1[:, k * P:(k + 1) * P], identity=ident_b[:]
                    )
                    nc.vector.tensor_copy(out=a1t[:, k, :], in_=pt1[:])
                ph = ps_ag.tile([P, Hdim], fp32, tag="hps", name="ph")
                for k in range(4):
                    nc.tensor.matmul(
                        out=ph[:], lhsT=a1t[:, k, :], rhs=wg1_s[:, k, :],
                        start=(k == 0), stop=False,
                    )
                nc.tensor.matmul(
                    out=ph[:], lhsT=ones_row[:], rhs=bg1_s[:],
                    start=False, stop=True, skip_group_check=True,
                )
                h_sb = work.tile([P, Hdim], bf16, tag="h_sb", name="h_sb")
                nc.scalar.activation(out=h_sb[:], in_=ph[:], func=AF.Relu)
                nc.sync.dma_start(out=h_own[i * P:(i + 1) * P, :], in_=h_sb[:])

            CH1 = CPC // 2  # first 5 chunks, then AllGather half 1
            with tc.tile_pool(name="ps_ag", bufs=2, space="PSUM") as ps_ag:
                for i in range(CH1):
                    l1_chunk(i, ps_ag)
                nc.gpsimd.collective_compute(
                    "AllGather", mybir.AluOpType.bypass, replica_groups=rg,
                    ins=[h_own[0:CH1 * P, :]], outs=[h_t[0:NPAD // 2, :]],
                )
                for i in range(CH1, CPC):
                    l1_chunk(i, ps_ag)
                nc.gpsimd.collective_compute(
                    "AllGather", mybir.AluOpType.bypass, replica_groups=rg,
                    ins=[h_own[CH1 * P:NPC, :]], outs=[h_t[NPAD // 2:NPAD, :]],
                )

            # ====== image MLP (emitted here so it fills the AllGather bubble) =
            with tc.tile_pool(name="ps_mlp", bufs=2, space="PSUM") as ps_mlp:
                for n in range(NT):
                    imt_n = work.tile([P, 4, 512], bf16, tag="imt_n")
                    nc.sync.dma_start(out=imt_n[:], in_=t_imt[n])
                    h1t = work.tile([P, 2, 512], bf16, tag="h1t")
                    for m in range(2):
                        pm = ps_mlp.tile([P, 512], fp32, tag="mlp1")
                        for k in range(4):
                            nc.tensor.matmul(
                                out=pm[:], lhsT=wi1_s[:, k, m, :], rhs=imt_n[:, k, :],
                                start=(k == 0), stop=(k == 3),
                            )
                        nc.scalar.activation(
                            out=h1t[:, m, :], in_=pm[:], func=AF.Relu,
                            bias=bi1_s[:, m:m + 1], scale=1.0,
                        )
                    for m in range(4):
                        pm2 = ps_mlp.tile([P, 512], fp32, tag="mlp2")
                        for k in range(2):
                            nc.tensor.matmul(
                                out=pm2[:], lhsT=wi2_s[:, k, m, :], rhs=h1t[:, k, :],
                                start=(k == 0), stop=(k == 1),
                            )
                        nc.scalar.activation(
                            out=imgT_s[:, m, n * 512:(n + 1) * 512], in_=pm2[:],
                            func=AF.Relu, bias=bi2_s[:, m:m + 1], scale=1.0,
                        )

            # ================= GCN layer 2 (my 10 chunks) =====================
            with tc.tile_pool(name="ps_l2", bufs=2, space="PSUM") as ps_l2:
                for i in range(CPC):
                    cm2 = gbuf.tile([P, T_MAX, P], bf16, tag="cm", name="cm2")
                    nc.scalar.dma_start(out=cm2[:], in_=t_cmat[:, i, :, :])
                    pa2 = ps_l2.tile([P, Hdim], fp32, tag="agg2", name="pa2")
                    g2s = []
                    for hf, (j0, j1) in enumerate(HALF):
                        g2h = gbuf.tile([P, j1 - j0, Hdim], bf16, tag=f"g2_{hf}", name="g2h")
                        nc.gpsimd.dma_gather(
                            out_ap=g2h[:], in_ap=h_t[:, :],
                            idxs_ap=gidx2_s[:, i, j0 * 8:j1 * 8],
                            num_idxs=(j1 - j0) * P, num_idxs_reg=(j1 - j0) * P,
                            elem_size=Hdim, single_packet=False,
                            queue_num=(2 * i + hf) % 4,
                        )
                        g2s.append(g2h)
                    for hf, (j0, j1) in enumerate(HALF):
                        for j in range(j0, j1):
                            nc.tensor.matmul(
                                out=pa2[:], lhsT=cm2[:, j, :], rhs=g2s[hf][:, j - j0, :],
                                start=(j == 0), stop=(j == T_MAX - 1),
                            )
                    a2 = work.tile([P, Hdim], bf16, tag="a2")
                    nc.vector.tensor_copy(out=a2[:], in_=pa2[:])
                    a2t = work.tile([P, 2, P], bf16, tag="a2t")
                    for k in range(2):
                        pt = ps_l2.tile([P, P], bf16, tag="tps")
                        nc.tensor.transpose(
                            out=pt[:], in_=a2[:, k * P:(k + 1) * P], identity=ident_b[:]
                        )
                        nc.vector.tensor_copy(out=a2t[:, k, :], in_=pt[:])
                    pg = ps_l2.tile([P, D], fp32, tag="outg")
                    for k in range(2):
                        nc.tensor.matmul(
                            out=pg[:], lhsT=a2t[:, k, :], rhs=wg2_s[:, k, :],
                            start=(k == 0), stop=False,
                        )
                    nc.tensor.matmul(
                        out=pg[:], lhsT=ones_row[:], rhs=bg2_s[:],
                        start=False, stop=True, skip_group_check=True,
                    )
                    g_sb = work.tile([P, D], bf16, tag="g_sb")
                    nc.vector.tensor_copy(out=g_sb[:], in_=pg[:])
                    nc.sync.dma_start(out=g_own[i * P:(i + 1) * P, :], in_=g_sb[:])

            # ================= txt gather (core-local) + transpose ============
            txt = big.tile([P, RT, D], bf16)
            nc.gpsimd.dma_gather(
                out_ap=txt[:], in_ap=g_own[:, :], idxs_ap=lidx_s[:],
                num_idxs=ROWS, num_idxs_reg=ROWS, elem_size=D,
                single_packet=False, queue_num=0,
            )
            txtT = big.tile([P, RT, 4, P], bf16)  # [p, r, k, rows]
            with tc.tile_pool(name="ps_tt", bufs=2, space="PSUM") as ps_tt:
                for r in range(RT):
                    for k in range(4):
                        ptt = ps_tt.tile([P, P], bf16, tag="ttps")
                        nc.tensor.transpose(
                            out=ptt[:], in_=txt[:, r, k * P:(k + 1) * P],
                            identity=ident_b[:],
                        )
                        nc.vector.tensor_copy(out=txtT[:, r, k, :], in_=ptt[:])

            # ================= logits + row losses ============================
            contrib = stat.tile([P, RT], fp32)
            with tc.tile_pool(name="ps_lg", bufs=8, space="PSUM") as ps_lg:
                for r in range(RT):
                    banks = []
                    for n in range(NT):
                        pl = ps_lg.tile([P, 512], fp32, tag="lg")
                        for k in range(4):
                            nc.tensor.matmul(
                                out=pl[:], lhsT=txtT[:, r, k, :],
                                rhs=imgT_s[:, k, n * 512:(n + 1) * 512],
                                start=(k == 0), stop=(k == 3),
                            )
                        banks.append(pl)
                    maxes = stat.tile([P, NT], fp32, tag="maxes")
                    for n in range(NT):
                        nc.vector.reduce_max(out=maxes[:, n:n + 1], in_=banks[n][:], axis=AX.X)
                    rmax = stat.tile([P, 1], fp32, tag="rmax")
                    nc.vector.reduce_max(out=rmax[:], in_=maxes[:], axis=AX.X)
                    nrmax = stat.tile([P, 1], fp32, tag="nrmax")
                    nc.scalar.mul(nrmax[:], rmax[:], -1.0)
                    # diag block for row-tile r is at local cols r*128..r*128+127
                    br, off = (r * P) // 512, (r * P) % 512
                    dtmp = stat.tile([P, P], fp32, tag="dtmp")
                    nc.vector.tensor_tensor(
                        out=dtmp[:], in0=banks[br][:, off:off + P], in1=ident_f[:],
                        op=mybir.AluOpType.mult,
                    )
                    diag = stat.tile([P, 1], fp32, tag="diag")
                    nc.vector.reduce_sum(out=diag[:], in_=dtmp[:], axis=AX.X)
                    sums = stat.tile([P, NT], fp32, tag="sums")
                    for n in range(NT):
                        esc = work.tile([P, 512], fp32, tag="esc")
                        nc.scalar.activation(
                            out=esc[:], in_=banks[n][:], func=AF.Exp,
                            bias=nrmax[:], scale=1.0, accum_out=sums[:, n:n + 1],
                        )
                    ssum = stat.tile([P, 1], fp32, tag="ssum")
                    nc.vector.reduce_sum(out=ssum[:], in_=sums[:], axis=AX.X)
                    lns = stat.tile([P, 1], fp32, tag="lns")
                    nc.scalar.activation(out=lns[:], in_=ssum[:], func=AF.Ln)
                    t1 = stat.tile([P, 1], fp32, tag="t1")
                    nc.vector.tensor_add(out=t1[:], in0=rmax[:], in1=lns[:])
                    nc.vector.tensor_sub(out=t1[:], in0=t1[:], in1=diag[:])
                    nc.vector.tensor_mul(
                        out=contrib[:, r:r + 1], in0=t1[:], in1=labf_s[:, r:r + 1]
                    )
            rsum = stat.tile([P, 1], fp32, tag="rsum")
            nc.vector.reduce_sum(out=rsum[:], in_=contrib[:], axis=AX.X)
            with tc.tile_pool(name="ps_fin", bufs=1, space="PSUM") as ps_fin:
                pf = ps_fin.tile([1, 1], fp32)
                nc.tensor.matmul(out=pf[:], lhsT=rsum[:], rhs=ones_col[:], start=True, stop=True)
                fin = stat.tile([1, 1], fp32, tag="fin")
                nc.vector.tensor_copy(out=fin[:], in_=pf[:])
            nc.sync.dma_start(out=t_out[:], in_=fin[:])

    nc.compile()
    return nc


_CACHE = {}


def kernel(**inputs) -> np.ndarray:
    from concourse.bass_utils import run_bass_kernel_spmd

    shared, percore, T_MAX, RT = _prep(inputs)
    key = (T_MAX, RT)
    if key not in _CACHE:
        _CACHE[key] = _build(T_MAX, RT)
    nc = _CACHE[key]

    in_maps = []
    for c in range(NCORES):
        m = {
            "xrow": shared["xrow"], "wg1": shared["wg1"], "wg2": shared["wg2"],
            "wi1": shared["wi1"], "wi2": shared["wi2"],
            "bg1": shared["bg1"], "bg2": shared["bg2"],
            "bi1": shared["bi1"], "bi2": shared["bi2"],
            "cmat": percore[c]["cmat"], "gidx": percore[c]["gidx"],
            "gidx2": percore[c]["gidx2"], "lidx": percore[c]["lidx"],
            "labf": percore[c]["labf"], "imt": percore[c]["imt"],
        }
        in_maps.append(m)

    trace = bool(int(os.environ.get("KERNEL_TRACE", "0")))
    try:
        res = run_bass_kernel_spmd(nc, in_maps, core_ids=list(range(NCORES)), trace=trace)
    except Exception:
        # transient NRT/device hiccups have been observed to clear on retry
        res = run_bass_kernel_spmd(nc, in_maps, core_ids=list(range(NCORES)), trace=trace)
    kernel.last_results = res
    total = sum(float(r["partial"][0, 0]) for r in res.results)
    return np.float32(total / BATCH + 1.0)
